# revision 8
# baseline (speedup 1.0000x reference)
"""3-layer GCN (GCNConv x3) on 8 TRN2 NeuronCores via Bass/Tile.

Math: per layer  out = A_hat @ (x @ W) + b  with A_hat = D^-1/2 (A+I) D^-1/2.
By linearity we aggregate first (Z = A_hat_w @ x as dense matmuls against
one-hot selection tiles), then h = Z @ W + b (+relu).

Sharding: 1D node partition, 8 cores x 49 dst-blocks x 128 nodes = 50176.
Source features for layers 2/3 are replicated via AllGather between layers.

Hot-path design:
- fp16 messages/S-tiles/weights; fp32 PSUM accumulate.
- Edge gathers for layers 2/3 are batched per SEGMENT (~16k source rows)
  through the SWDGE dma_gather path (out[p,c,:] = table[idx16[c*128+p],:]),
  amortizing the ~1us SWDGE fixed cost ~8000x vs per-chunk indirect DMAs.
  dma_gather indices are int16, so each block's edge slots are split into
  a low run (src < 32768, gathered from the table base) and a high run
  (gathered from table row 32768 on); each run is padded to a whole
  128-slot chunk.
- Selection tiles S[e, d] = w_e * (d == dloc_e) are built on-chip from
  per-edge (dloc, w) columns, split across three engines so the PE is
  never starved: DVE fused iota-compare-multiply, Activation via
  relu(w - w*(d - dloc)^2), and a GpSimd share of the DVE form.
- Per-block tail (PSUM->SBUF copy, W GEMM, bias, relu, writeback) is
  software-pipelined one block behind the aggregation matmuls so the PE
  never stalls waiting on the scalar-engine PSUM copy.
"""

import numpy as np

N = 50000
D = 128
P = 128
NCORES = 8
BLK = 49                  # dst blocks per core
PER = BLK * P             # 6272 nodes per core
NPAD = NCORES * PER       # 50176
CAP = 140                 # max chunk columns per gather/stream segment
TSPLIT = 32768            # int16 index split point for dma_gather

_CACHE = {}


def _prep_graph(edge_index):
    """Host index preprocessing: bucket edges by dst block, split each
    block's edges into low/high source halves (dma_gather int16 indices),
    pack per-slot (src, dloc, w) arrays in slot-major order.

    Slot layout: segments of <= CAP chunk columns; within a segment, first
    the low-src chunks of its blocks, then the high-src chunks. Padding
    slots gather row 0 with w = 0 so they contribute nothing. Chunk counts
    are shared across cores (SPMD: one instruction stream).
    """
    src = np.concatenate([edge_index[0].astype(np.int64),
                          np.arange(N, dtype=np.int64)])
    dst = np.concatenate([edge_index[1].astype(np.int64),
                          np.arange(N, dtype=np.int64)])
    deg = np.bincount(dst, minlength=N).astype(np.float64)
    dinv = (1.0 / np.sqrt(deg)).astype(np.float32)
    w = (dinv[src] * dinv[dst]).astype(np.float32)

    # order edges by (block, is_high_src) so each block is one low run
    # followed by one high run
    gblk = dst // P
    is_hi = (src >= TSPLIT).astype(np.int64)
    order = np.lexsort((is_hi, gblk))
    src, dst, w, gblk, is_hi = (src[order], dst[order], w[order],
                                gblk[order], is_hi[order])

    nblk = NCORES * BLK
    cnt_lo = np.bincount(gblk[is_hi == 0], minlength=nblk).reshape(NCORES, BLK)
    cnt_hi = np.bincount(gblk[is_hi == 1], minlength=nblk).reshape(NCORES, BLK)
    Klo = ((cnt_lo + P - 1) // P).max(axis=0)
    Khi = ((cnt_hi + P - 1) // P).max(axis=0)
    Klo[(Klo + Khi) == 0] = 1

    # greedy segment packing: whole blocks, at most CAP chunk columns each
    seg_blocks = []
    cur, cur_cols = [], 0
    for b in range(BLK):
        kb = int(Klo[b] + Khi[b])
        if cur and cur_cols + kb > CAP:
            seg_blocks.append(cur)
            cur, cur_cols = [], 0
        cur.append(b)
        cur_cols += kb
    seg_blocks.append(cur)

    # absolute slot bases, low run then high run per segment
    lo_base = np.zeros(BLK, np.int64)
    hi_base = np.zeros(BLK, np.int64)
    segs = []        # (S0, Wlo, Whi, [(b, lo_base, Klo, hi_base, Khi)...])
    S0 = 0
    for blocks in seg_blocks:
        Wlo = int(sum(Klo[b] for b in blocks))
        Whi = int(sum(Khi[b] for b in blocks))
        pos = S0
        for b in blocks:
            lo_base[b] = pos
            pos += Klo[b]
        for b in blocks:
            hi_base[b] = pos
            pos += Khi[b]
        segs.append((S0, Wlo, Whi,
                     [(int(b), int(lo_base[b]), int(Klo[b]),
                       int(hi_base[b]), int(Khi[b])) for b in blocks]))
        S0 = pos
    SCOLS = S0

    # per-edge slot placement: rank within (block, half) run
    run_id = gblk * 2 + is_hi
    run_counts = np.bincount(run_id, minlength=2 * nblk)
    run_starts = np.concatenate([[0], np.cumsum(run_counts)])
    r = np.arange(len(dst)) - run_starts[run_id]
    core = gblk // BLK
    b_loc = gblk % BLK
    base = np.where(is_hi == 1, hi_base[b_loc], lo_base[b_loc])
    sub = base + r // P
    lane = r % P

    srcfull = np.zeros((NCORES, P, SCOLS), np.int32)
    srcfull[core, lane, sub] = src
    val16 = np.zeros((NCORES, P, SCOLS), np.int16)
    val16[core, lane, sub] = (src - is_hi * TSPLIT).astype(np.int16)
    dloc = np.zeros((NCORES, P, SCOLS), np.float32)
    dloc[core, lane, sub] = (dst % P).astype(np.float32)
    wv = np.zeros((NCORES, P, SCOLS), np.float32)
    wv[core, lane, sub] = w

    # wrapped int16 index layout for dma_gather: entry for (p, slot) at
    # [p % 16, slot*8 + p//16], replicated to 128 partitions
    a = val16.transpose(0, 2, 1).reshape(NCORES, SCOLS, 8, 16)
    idx16 = np.ascontiguousarray(
        a.transpose(0, 3, 1, 2).reshape(NCORES, 16, SCOLS * 8))
    idx16 = np.tile(idx16, (1, 8, 1))

    key = (tuple(int(k) for k in Klo), tuple(int(k) for k in Khi))
    return srcfull, idx16, dloc, wv, key, segs


def _build(key, segs):
    import concourse.mybir as mybir
    import concourse.tile as tile
    from concourse import bacc

    f16 = mybir.dt.float16
    f32 = mybir.dt.float32
    SCOLS = sum(key[0]) + sum(key[1])

    nc = bacc.Bacc("TRN2", target_bir_lowering=False, debug=False,
                   num_devices=NCORES)

    mx_in = nc.dram_tensor("mx", [P, SCOLS * P], f16,
                           kind="ExternalInput").ap()
    idx16_in = nc.dram_tensor("idx16", [P, SCOLS * 8], mybir.dt.int16,
                              kind="ExternalInput").ap()
    dloc_in = nc.dram_tensor("dloc", [P, SCOLS], f32,
                             kind="ExternalInput").ap()
    ndloc_in = nc.dram_tensor("ndloc", [P, SCOLS], f32,
                              kind="ExternalInput").ap()
    w_in = nc.dram_tensor("wv", [P, SCOLS], f32, kind="ExternalInput").ap()
    nw_in = nc.dram_tensor("nwv", [P, SCOLS], f32, kind="ExternalInput").ap()
    Ws = [nc.dram_tensor(f"W{l}", [D, D], f16, kind="ExternalInput").ap()
          for l in (1, 2, 3)]
    bs = [nc.dram_tensor(f"b{l}", [1, D], f16, kind="ExternalInput").ap()
          for l in (1, 2, 3)]
    out = nc.dram_tensor("out", [PER, D], f32, kind="ExternalOutput").ap()

    with tile.TileContext(nc) as tc:
        with tc.tile_pool(name="const", bufs=1) as cpool, \
             tc.tile_pool(name="msg", bufs=3) as mpool, \
             tc.tile_pool(name="sel", bufs=10) as spool, \
             tc.tile_pool(name="t2", bufs=4) as tpool, \
             tc.tile_pool(name="zsb", bufs=3) as zpool, \
             tc.tile_pool(name="hsb", bufs=3) as hpool, \
             tc.tile_pool(name="pz", bufs=4, space="PSUM") as pz, \
             tc.tile_pool(name="ph", bufs=4, space="PSUM") as ph, \
             tc.tile_pool(name="dram", bufs=1, space="DRAM") as dram:

            iota32 = cpool.tile([P, P], f32)
            nc.gpsimd.iota(iota32[:], pattern=[[1, P]], base=0,
                           channel_multiplier=0,
                           allow_small_or_imprecise_dtypes=True)
            ones_t = cpool.tile([1, P], f16, name="ones")
            nc.vector.memset(ones_t[:], 1.0)

            w_t, b_t = [], []
            for l in range(3):
                wt = cpool.tile([D, D], f16, name=f"wt{l}")
                nc.sync.dma_start(out=wt[:], in_=Ws[l][:])
                bt = cpool.tile([1, D], f16, name=f"bt{l}")
                nc.sync.dma_start(out=bt[:], in_=bs[l][:])
                w_t.append(wt)
                b_t.append(bt)
            idx16_sb = cpool.tile([P, SCOLS * 8], mybir.dt.int16, name="idx16")
            nc.sync.dma_start(out=idx16_sb[:], in_=idx16_in[:])
            dloc_sb = cpool.tile([P, SCOLS], f32, name="dloc")
            nc.sync.dma_start(out=dloc_sb[:], in_=dloc_in[:])
            ndloc_sb = cpool.tile([P, SCOLS], f32, name="ndloc")
            nc.sync.dma_start(out=ndloc_sb[:], in_=ndloc_in[:])
            wv_sb = cpool.tile([P, SCOLS], f32, name="wv")
            nc.sync.dma_start(out=wv_sb[:], in_=w_in[:])
            nwv_sb = cpool.tile([P, SCOLS], f32, name="nwv")
            nc.sync.dma_start(out=nwv_sb[:], in_=nw_in[:])

            h_full = [None, None]
            ag_in = [None, None]
            for l in range(2):
                ag_in[l] = dram.tile([PER, D], f16, name=f"ag_in{l}")
                h_full[l] = dram.tile([NPAD, D], f16, addr_space="Shared",
                                      name=f"h_full{l}")

            def build_s(c):
                """Emit the selection-tile build for chunk column c on an
                engine chosen by c%8; returns the tile."""
                s_t = spool.tile([P, P], f16, tag="s")
                r = c & 7
                if r in (5, 6):
                    # Activation engine: relu(w - w*(d - dloc)^2)
                    t2 = tpool.tile([P, P], f16, tag="t2")
                    nc.scalar.activation(
                        t2[:], iota32[:],
                        mybir.ActivationFunctionType.Square,
                        bias=ndloc_sb[:, c:c + 1])
                    nc.scalar.activation(
                        s_t[:], t2[:],
                        mybir.ActivationFunctionType.Relu,
                        bias=wv_sb[:, c:c + 1],
                        scale=nwv_sb[:, c:c + 1])
                else:
                    eng = nc.gpsimd if r == 7 else nc.vector
                    eng.tensor_scalar(
                        out=s_t[:], in0=iota32[:],
                        scalar1=dloc_sb[:, c:c + 1],
                        scalar2=wv_sb[:, c:c + 1],
                        op0=mybir.AluOpType.is_equal,
                        op1=mybir.AluOpType.mult)
                return s_t

            pending = [None]    # deferred tail: (zt_sb, block_idx, layer)

            def flush_tail():
                if pending[0] is None:
                    return
                zt_sb, b, l = pending[0]
                pending[0] = None
                h_ps = ph.tile([P, P], f32, space="PSUM", tag="h")
                nc.tensor.matmul(out=h_ps[:], lhsT=zt_sb[:], rhs=w_t[l][:],
                                 start=True, stop=False)
                nc.tensor.matmul(out=h_ps[:], lhsT=ones_t[:], rhs=b_t[l][:],
                                 start=False, stop=True)
                if l < 2:
                    h_sb = hpool.tile([P, P], f16, tag="hs")
                    nc.scalar.activation(h_sb[:], h_ps[:],
                                         mybir.ActivationFunctionType.Relu)
                    nc.sync.dma_start(out=ag_in[l][b * P:(b + 1) * P, :],
                                      in_=h_sb[:])
                else:
                    h_sb = hpool.tile([P, P], f32, tag="ho")
                    nc.scalar.activation(h_sb[:], h_ps[:],
                                         mybir.ActivationFunctionType.Copy)
                    nc.sync.dma_start(out=out[b * P:(b + 1) * P, :],
                                      in_=h_sb[:])

            for l in range(3):
                for (S0, Wlo, Whi, blocks) in segs:
                    W = Wlo + Whi
                    m_t = mpool.tile([P, CAP * P], f16, tag="m")
                    if l == 0:
                        nc.sync.dma_start(out=m_t[:, :W * P],
                                          in_=mx_in[:, S0 * P:(S0 + W) * P])
                    else:
                        # SWDGE descriptor carveout is ~16KB; keep each
                        # dma_gather under 1024 descriptors (7 chunks).
                        SUB = 7
                        for (qa, qb, tbl) in ((0, Wlo, h_full[l - 1][:]),
                                              (Wlo, W,
                                               h_full[l - 1][TSPLIT:, :])):
                            for q0 in range(qa, qb, SUB):
                                qn = min(SUB, qb - q0)
                                nc.gpsimd.dma_gather(
                                    out_ap=m_t[:, q0 * P:(q0 + qn) * P]
                                    .rearrange("p (c f) -> p c f", f=P),
                                    in_ap=tbl,
                                    idxs_ap=idx16_sb[:, (S0 + q0) * 8:
                                                     (S0 + q0 + qn) * 8],
                                    num_idxs=qn * P,
                                    num_idxs_reg=qn * P,
                                    elem_size=P)
                    for (b, lob, klo, hib, khi) in blocks:
                        slots = ([lob + k for k in range(klo)]
                                 + [hib + k for k in range(khi)])
                        zt = pz.tile([P, P], f32, space="PSUM", tag="zt")
                        for t, c in enumerate(slots):
                            cc = c - S0
                            s_t = build_s(c)
                            nc.tensor.matmul(
                                out=zt[:], lhsT=m_t[:, cc * P:(cc + 1) * P],
                                rhs=s_t[:], start=(t == 0),
                                stop=(t == len(slots) - 1))
                        zt_sb = zpool.tile([P, P], f16, tag="z")
                        nc.scalar.activation(
                            zt_sb[:], zt[:], mybir.ActivationFunctionType.Copy)
                        flush_tail()
                        pending[0] = (zt_sb, b, l)
                flush_tail()
                if l < 2:
                    nc.gpsimd.collective_compute(
                        "AllGather", mybir.AluOpType.bypass,
                        replica_groups=[list(range(NCORES))],
                        ins=[ag_in[l].opt()], outs=[h_full[l].opt()],
                    )

    nc.compile()
    return nc


def _get_compiled(key, segs):
    if key not in _CACHE:
        _CACHE[key] = _build(key, segs)
    return _CACHE[key]


def _make_in_maps(x, edge_index, W1, b1, W2, b2, W3, b3):
    srcfull, idx16, dloc, wv, key, segs = _prep_graph(np.asarray(edge_index))
    SCOLS = sum(key[0]) + sum(key[1])
    x_pad = np.zeros((NPAD, D), np.float16)
    x_pad[:N] = np.asarray(x, np.float32).astype(np.float16)
    common = {
        "W1": np.asarray(W1, np.float32).astype(np.float16),
        "b1": np.asarray(b1, np.float32).astype(np.float16).reshape(1, D),
        "W2": np.asarray(W2, np.float32).astype(np.float16),
        "b2": np.asarray(b2, np.float32).astype(np.float16).reshape(1, D),
        "W3": np.asarray(W3, np.float32).astype(np.float16),
        "b3": np.asarray(b3, np.float32).astype(np.float16).reshape(1, D),
    }
    in_maps = []
    for c in range(NCORES):
        m = dict(common)
        m["idx16"] = idx16[c]
        m["dloc"] = dloc[c]
        m["ndloc"] = -dloc[c]
        m["wv"] = wv[c]
        m["nwv"] = -wv[c]
        m["mx"] = x_pad[srcfull[c]].reshape(P, SCOLS * D)
        in_maps.append(m)
    return in_maps, key, segs


def _install_profile_shim():
    """This image's antenv lacks axon_hooks; recreate the NTFF hook from
    the boot helper so trace=True works. Test-side only."""
    import sys
    import types
    try:
        import antenv.axon_hooks  # noqa: F401
        return
    except ImportError:
        pass
    sys.path.insert(0, "/root/.axon_site/trn_agent_boot")
    import trn_boot
    hook = trn_boot._ntff_profile_via_ctypes("/opt/axon/libaxon_pjrt.so")
    import antenv
    mod = types.ModuleType("antenv.axon_hooks")
    state = {"hook": hook}
    mod.get_axon_ntff_profile_hook = lambda: state["hook"]
    mod.set_axon_ntff_profile_hook = lambda h: state.update(hook=h)
    sys.modules["antenv.axon_hooks"] = mod
    antenv.axon_hooks = mod
    # no fish credentials in this container; keep artifacts local
    import concourse.bass_utils as bu
    bu.upload_artifacts = lambda tmpdir: "local://" + str(tmpdir)


def _run(in_maps, key, segs, trace=False, tmpdir=None):
    from concourse.bass_utils import run_bass_kernel_spmd
    if trace:
        _install_profile_shim()
    nc = _get_compiled(key, segs)
    res = run_bass_kernel_spmd(nc, in_maps, core_ids=list(range(NCORES)),
                               trace=trace, tmpdir=tmpdir)
    return res


def kernel(x, edge_index, W1, b1, W2, b2, W3, b3):
    in_maps, key, segs = _make_in_maps(x, edge_index, W1, b1, W2, b2, W3, b3)
    res = _run(in_maps, key, segs)
    parts = [res.results[c]["out"] for c in range(NCORES)]
    return np.concatenate(parts, axis=0)[:N].astype(np.float32)


def kernel_profiled(x, edge_index, W1, b1, W2, b2, W3, b3, tmpdir=None):
    """Like kernel() but runs with NTFF tracing; returns (output, results)."""
    in_maps, key, segs = _make_in_maps(x, edge_index, W1, b1, W2, b2, W3, b3)
    res = _run(in_maps, key, segs, trace=True, tmpdir=tmpdir)
    parts = [res.results[c]["out"] for c in range(NCORES)]
    return np.concatenate(parts, axis=0)[:N].astype(np.float32), res


# revision 9
# speedup vs baseline: 1.1243x; 1.1243x over previous
"""3-layer GCN (GCNConv x3) on 8 TRN2 NeuronCores via Bass/Tile.

Math: per layer  out = A_hat @ (x @ W) + b  with A_hat = D^-1/2 (A+I) D^-1/2.
By linearity we aggregate first (Z = A_hat_w @ x as dense matmuls against
one-hot selection tiles), then h = Z @ W + b (+relu).

Sharding: 1D node partition, 8 cores x 49 dst-blocks x 128 nodes = 50176.
Source features for layers 2/3 are replicated via AllGather between layers.

Hot-path design (vs the naive per-chunk indirect-DMA kernel):
- fp16 end to end (gather table, messages, S tiles, weights); fp32 PSUM.
- Edge gathers batched: ONE indirect_dma_start per 7-block group moves
  16k source rows (offset AP [128, chunks], out [128, chunks*128]),
  amortizing the ~1us SWDGE fixed cost ~126x vs per-chunk gathers.
- Selection tiles S[e, d] = w_e * (d == dloc_e) are built on-chip by the
  (otherwise idle) vector engine from per-edge (dloc, w) columns via a fused
  iota-compare-multiply, instead of streaming dense one-hot tiles from HBM.
- The per-block GEMM uses Z^T as the stationary operand so h = Z @ W + 1*b^T
  comes out node-major; no transpose is needed before the table write.
"""

import numpy as np

N = 50000
D = 128
P = 128
NCORES = 8
BLK = 49                  # dst blocks per core
PER = BLK * P             # 6272 nodes per core
NPAD = NCORES * PER       # 50176
GRP = 7                   # dst blocks per gather group
NGRP = BLK // GRP         # 7 groups per core

_CACHE = {}


def _prep_graph(edge_index):
    """Host index preprocessing: sort edges by dst, pack per-slot gather
    indices plus per-slot (dloc, w) selection data.

    Slot layout: chunk col = b_loc*K + j//128, lane = j%128 for the j-th
    edge of block b_loc (sorted by dst). Padding slots gather row 0 with
    w = 0 so they contribute nothing.

    Returns (idx32[NC, 128, BLK*K], dloc[NC, 128, BLK*K], wv[same], K).
    """
    src = np.concatenate([edge_index[0].astype(np.int64),
                          np.arange(N, dtype=np.int64)])
    dst = np.concatenate([edge_index[1].astype(np.int64),
                          np.arange(N, dtype=np.int64)])
    deg = np.bincount(dst, minlength=N).astype(np.float64)
    dinv = (1.0 / np.sqrt(deg)).astype(np.float32)
    w = (dinv[src] * dinv[dst]).astype(np.float32)

    order = np.argsort(dst, kind="stable")
    src, dst, w = src[order], dst[order], w[order]

    nblk = NCORES * BLK
    gblk = dst // P
    counts = np.bincount(gblk, minlength=nblk)
    block_starts = np.concatenate([[0], np.cumsum(counts)])
    K = int(np.ceil(counts.max() / P))

    j = np.arange(len(dst)) - block_starts[gblk]     # rank within block
    core = gblk // BLK
    b_loc = gblk % BLK
    sub = b_loc * K + j // P                         # chunk col within core
    lane = j % P

    idx32 = np.zeros((NCORES, P, BLK * K), np.int32)
    idx32[core, lane, sub] = src
    dloc = np.zeros((NCORES, P, BLK * K), np.float32)
    dloc[core, lane, sub] = (dst % P).astype(np.float32)
    wv = np.zeros((NCORES, P, BLK * K), np.float32)
    wv[core, lane, sub] = w
    return idx32, dloc, wv, K


def _build(K):
    import concourse.bass as bass
    import concourse.mybir as mybir
    import concourse.tile as tile
    from concourse import bacc

    f16 = mybir.dt.float16
    f32 = mybir.dt.float32
    SCOLS = BLK * K             # chunk columns per core
    GCOLS = GRP * K             # chunk columns per gather group

    nc = bacc.Bacc("TRN2", target_bir_lowering=False, debug=False,
                   num_devices=NCORES)

    x_pad = nc.dram_tensor("x_pad", [NPAD, D], f16, kind="ExternalInput").ap()
    mx_in = nc.dram_tensor("mx", [NGRP * P, GCOLS * P], f16,
                           kind="ExternalInput").ap()
    idx_in = nc.dram_tensor("idx", [P, SCOLS], mybir.dt.int32,
                            kind="ExternalInput").ap()
    dloc_in = nc.dram_tensor("dloc", [P, SCOLS], f32,
                             kind="ExternalInput").ap()
    w_in = nc.dram_tensor("wv", [P, SCOLS], f32, kind="ExternalInput").ap()
    Ws = [nc.dram_tensor(f"W{l}", [D, D], f16, kind="ExternalInput").ap()
          for l in (1, 2, 3)]
    bs = [nc.dram_tensor(f"b{l}", [1, D], f16, kind="ExternalInput").ap()
          for l in (1, 2, 3)]
    out = nc.dram_tensor("out", [PER, D], f32, kind="ExternalOutput").ap()

    with tile.TileContext(nc) as tc:
        with tc.tile_pool(name="const", bufs=1) as cpool, \
             tc.tile_pool(name="msg", bufs=2) as mpool, \
             tc.tile_pool(name="msg1", bufs=32) as m1pool, \
             tc.tile_pool(name="sel", bufs=16) as spool, \
             tc.tile_pool(name="work", bufs=4) as wpool, \
             tc.tile_pool(name="pz", bufs=3, space="PSUM") as pz, \
             tc.tile_pool(name="ph", bufs=3, space="PSUM") as ph, \
             tc.tile_pool(name="dram", bufs=1, space="DRAM") as dram:

            iota_t = cpool.tile([P, P], f32)
            nc.gpsimd.iota(iota_t[:], pattern=[[1, P]], base=0,
                           channel_multiplier=0,
                           allow_small_or_imprecise_dtypes=True)
            ones_t = cpool.tile([1, P], f16, name="ones")
            nc.vector.memset(ones_t[:], 1.0)

            w_t, b_t = [], []
            for l in range(3):
                wt = cpool.tile([D, D], f16, name=f"wt{l}")
                nc.sync.dma_start(out=wt[:], in_=Ws[l][:])
                bt = cpool.tile([1, D], f16, name=f"bt{l}")
                nc.sync.dma_start(out=bt[:], in_=bs[l][:])
                w_t.append(wt)
                b_t.append(bt)
            idx_sb = cpool.tile([P, SCOLS], mybir.dt.int32, name="idx")
            nc.sync.dma_start(out=idx_sb[:], in_=idx_in[:])
            dloc_sb = cpool.tile([P, SCOLS], f32, name="dloc")
            nc.sync.dma_start(out=dloc_sb[:], in_=dloc_in[:])
            wv_sb = cpool.tile([P, SCOLS], f32, name="wv")
            nc.sync.dma_start(out=wv_sb[:], in_=w_in[:])

            h_full = [None, None]
            ag_in = [None, None]
            for l in range(2):
                ag_in[l] = dram.tile([PER, D], f16, name=f"ag_in{l}")
                h_full[l] = dram.tile([NPAD, D], f16, addr_space="Shared",
                                      name=f"h_full{l}")

            for l in range(3):
                table = x_pad if l == 0 else h_full[l - 1][:]
                for g in range(NGRP):
                    if l == 0:
                        m_t = mpool.tile([P, GCOLS * P], f16, tag="m")
                        nc.sync.dma_start(out=m_t[:],
                                          in_=mx_in[g * P:(g + 1) * P, :])
                    for b_loc in range(GRP):
                        b = g * GRP + b_loc
                        zt = pz.tile([P, P], f32, space="PSUM", tag="zt")
                        for k in range(K):
                            col = b * K + k
                            if l == 0:
                                m_ap = m_t[:, (b_loc * K + k) * P:
                                           (b_loc * K + k + 1) * P]
                            else:
                                m1 = m1pool.tile([P, P], f16, tag="m1")
                                nc.gpsimd.indirect_dma_start(
                                    out=m1[:], out_offset=None, in_=table,
                                    in_offset=bass.IndirectOffsetOnAxis(
                                        ap=idx_sb[:, col:col + 1], axis=0),
                                )
                                m_ap = m1[:]
                            s_t = spool.tile([P, P], f16, tag="s")
                            nc.vector.tensor_scalar(
                                out=s_t[:], in0=iota_t[:],
                                scalar1=dloc_sb[:, col:col + 1],
                                scalar2=wv_sb[:, col:col + 1],
                                op0=mybir.AluOpType.is_equal,
                                op1=mybir.AluOpType.mult)
                            nc.tensor.matmul(out=zt[:], lhsT=m_ap, rhs=s_t[:],
                                             start=(k == 0),
                                             stop=(k == K - 1))
                        zt_sb = wpool.tile([P, P], f16, tag="z")
                        nc.scalar.activation(
                            zt_sb[:], zt[:], mybir.ActivationFunctionType.Copy)
                        h_ps = ph.tile([P, P], f32, space="PSUM", tag="h")
                        nc.tensor.matmul(out=h_ps[:], lhsT=zt_sb[:],
                                         rhs=w_t[l][:], start=True, stop=False)
                        nc.tensor.matmul(out=h_ps[:], lhsT=ones_t[:],
                                         rhs=b_t[l][:], start=False, stop=True)
                        if l < 2:
                            h_sb = wpool.tile([P, P], f16, tag="hs")
                            nc.scalar.activation(
                                h_sb[:], h_ps[:],
                                mybir.ActivationFunctionType.Relu)
                            nc.sync.dma_start(
                                out=ag_in[l][b * P:(b + 1) * P, :],
                                in_=h_sb[:])
                        else:
                            h_sb = wpool.tile([P, P], f32, tag="ho")
                            nc.vector.tensor_copy(out=h_sb[:], in_=h_ps[:])
                            nc.sync.dma_start(
                                out=out[b * P:(b + 1) * P, :], in_=h_sb[:])
                if l < 2:
                    nc.gpsimd.collective_compute(
                        "AllGather", mybir.AluOpType.bypass,
                        replica_groups=[list(range(NCORES))],
                        ins=[ag_in[l].opt()], outs=[h_full[l].opt()],
                    )

    nc.compile()
    return nc


def _get_compiled(K):
    if K not in _CACHE:
        _CACHE[K] = _build(K)
    return _CACHE[K]


def _make_in_maps(x, edge_index, W1, b1, W2, b2, W3, b3):
    idx32, dloc, wv, K = _prep_graph(np.asarray(edge_index))
    x_pad = np.zeros((NPAD, D), np.float16)
    x_pad[:N] = np.asarray(x, np.float32).astype(np.float16)
    common = {
        "x_pad": x_pad,
        "W1": np.asarray(W1, np.float32).astype(np.float16),
        "b1": np.asarray(b1, np.float32).astype(np.float16).reshape(1, D),
        "W2": np.asarray(W2, np.float32).astype(np.float16),
        "b2": np.asarray(b2, np.float32).astype(np.float16).reshape(1, D),
        "W3": np.asarray(W3, np.float32).astype(np.float16),
        "b3": np.asarray(b3, np.float32).astype(np.float16).reshape(1, D),
    }
    GCOLS = GRP * K
    in_maps = []
    for c in range(NCORES):
        m = dict(common)
        m["idx"] = idx32[c]
        m["dloc"] = dloc[c]
        m["wv"] = wv[c]
        rows = x_pad[idx32[c]]                     # [128, SCOLS, D]
        m["mx"] = rows.reshape(P, NGRP, GCOLS, D).transpose(1, 0, 2, 3) \
                      .reshape(NGRP * P, GCOLS * D)
        in_maps.append(m)
    return in_maps, K


def _install_profile_shim():
    """This image's antenv lacks axon_hooks; recreate the NTFF hook from
    the boot helper so trace=True works. Test-side only."""
    import sys
    import types
    try:
        import antenv.axon_hooks  # noqa: F401
        return
    except ImportError:
        pass
    sys.path.insert(0, "/root/.axon_site/trn_agent_boot")
    import trn_boot
    hook = trn_boot._ntff_profile_via_ctypes("/opt/axon/libaxon_pjrt.so")
    import antenv
    mod = types.ModuleType("antenv.axon_hooks")
    state = {"hook": hook}
    mod.get_axon_ntff_profile_hook = lambda: state["hook"]
    mod.set_axon_ntff_profile_hook = lambda h: state.update(hook=h)
    sys.modules["antenv.axon_hooks"] = mod
    antenv.axon_hooks = mod
    # no fish credentials in this container; keep artifacts local
    import concourse.bass_utils as bu
    bu.upload_artifacts = lambda tmpdir: "local://" + str(tmpdir)


def _run(in_maps, K, trace=False, tmpdir=None):
    from concourse.bass_utils import run_bass_kernel_spmd
    if trace:
        _install_profile_shim()
    nc = _get_compiled(K)
    res = run_bass_kernel_spmd(nc, in_maps, core_ids=list(range(NCORES)),
                               trace=trace, tmpdir=tmpdir)
    return res


def kernel(x, edge_index, W1, b1, W2, b2, W3, b3):
    in_maps, K = _make_in_maps(x, edge_index, W1, b1, W2, b2, W3, b3)
    res = _run(in_maps, K)
    parts = [res.results[c]["out"] for c in range(NCORES)]
    return np.concatenate(parts, axis=0)[:N].astype(np.float32)


def kernel_profiled(x, edge_index, W1, b1, W2, b2, W3, b3, tmpdir=None):
    """Like kernel() but runs with NTFF tracing; returns (output, results)."""
    in_maps, K = _make_in_maps(x, edge_index, W1, b1, W2, b2, W3, b3)
    res = _run(in_maps, K, trace=True, tmpdir=tmpdir)
    parts = [res.results[c]["out"] for c in range(NCORES)]
    return np.concatenate(parts, axis=0)[:N].astype(np.float32), res

